# revision 1
# baseline (speedup 1.0000x reference)
"""Tensor-parallel MultiHeadAttention (GQA + RMSNorm-KV + RoPE) for 8 trn2 cores.

Sharding: KV head h -> core h (HKV=8); Q heads {2h, 2h+1}; x replicated;
Wo row-sharded; host sums the 8 partial outputs.
"""
import sys
sys.path.insert(0, '/opt/trn_rl_repo')
import numpy as np
import concourse.bass as bass
import concourse.tile as tile
from concourse import mybir
from contextlib import ExitStack

F32 = mybir.dt.float32
F32R = mybir.dt.float32r
AF = mybir.ActivationFunctionType

# Problem constants (full size). Overridable for mini testing.
B = 2
S = 2048          # sequence per batch
D = 4096          # model dim
HD = 256          # head dim
DQ = 512          # per-core q width (2 heads)
ROPE_BASE = 10000.0
MASKV = -1e10
EPS = 1e-6
N_CORES = 8


def legalize_waits(nc, max_waits=1):
    """This container's walrus encodes at most one sem-wait per instruction.
    Move extra waits onto same-engine NOPs placed just before (engine FIFO
    order makes that equivalent)."""
    n = 0
    for f in nc.m.functions:
        for blk in f.blocks:
            out = []
            for ins in blk.instructions:
                si = ins.sync_info
                if si is not None and si.on_wait and len(si.on_wait) > max_waits:
                    waits = list(si.on_wait)
                    for w in waits[max_waits:]:
                        nop = mybir.InstNoOp(name=nc.get_next_instruction_name())
                        nop.engine = ins.engine
                        nop.sync_info = mybir.SyncInfo(on_wait=[w], on_update=[])
                        out.append(nop)
                    ins.sync_info = mybir.SyncInfo(
                        on_wait=waits[:max_waits], on_update=list(si.on_update or []))
                    n += 1
                out.append(ins)
            blk.instructions.clear()
            for i in out:
                blk.instructions.append(i)
    return n


def build_bass(b=B, s=S, d=D, legalize=True, phases="ABCD"):
    T = b * s              # total tokens
    NF = d // 128          # contraction tiles
    CH = 128               # phase-A token chunk (x is stationary)
    NCH = T // CH
    TTB = s // 128         # token tiles per batch
    NQB = s // 512         # 512-wide query blocks per batch

    nc = bass.Bass()
    xT = nc.dram_tensor("xT", [d, T], F32R, kind="ExternalInput")
    wq = nc.dram_tensor("wq", [d, DQ], F32R, kind="ExternalInput")
    wkv = nc.dram_tensor("wkv", [d, 512], F32R, kind="ExternalInput")
    wo = nc.dram_tensor("wo", [DQ, d], F32R, kind="ExternalInput")
    cosd = nc.dram_tensor("cosd", [T, 128], F32, kind="ExternalInput")
    sind = nc.dram_tensor("sind", [T, 128], F32, kind="ExternalInput")
    kscd = nc.dram_tensor("kscd", [128, 256], F32, kind="ExternalInput")
    vscd = nc.dram_tensor("vscd", [128, 256], F32, kind="ExternalInput")
    identd = nc.dram_tensor("identd", [128, 128], F32R, kind="ExternalInput")
    onesd = nc.dram_tensor("onesd", [128, 128], F32R, kind="ExternalInput")
    maskd = nc.dram_tensor("maskd", [4, 128, 512], F32, kind="ExternalInput")
    y = nc.dram_tensor("y", [T, d], F32, kind="ExternalOutput")

    qTd = nc.dram_tensor("qTd", [DQ, T], F32R)
    kTd = nc.dram_tensor("kTd", [256, T], F32R)
    vd = nc.dram_tensor("vd", [T, 256], F32R)

    with tile.TileContext(nc) as tc, ExitStack() as top:
        cp = top.enter_context(tc.tile_pool(name="const", bufs=1))
        zero_b = cp.tile([128, 1], F32)
        nc.vector.memset(zero_b[:], 0.0)
        eps_b = cp.tile([128, 1], F32)
        nc.vector.memset(eps_b[:], EPS)
        ksc = cp.tile([128, 256], F32)
        nc.sync.dma_start(ksc[:], kscd[:])
        vsc = cp.tile([128, 256], F32)
        nc.sync.dma_start(vsc[:], vscd[:])
        ident = cp.tile([128, 128], F32R)
        nc.sync.dma_start(ident[:], identd[:])
        ones_sb = cp.tile([128, 128], F32R)
        nc.sync.dma_start(ones_sb[:], onesd[:])
        maskb = cp.tile([128, 4, 512], F32)
        nc.sync.dma_start(maskb[:], maskd[:].rearrange("m p t -> p m t"))

        # ---------------- Phase A: projections + norm + rope ----------------
        with ExitStack() as pa:
            wp = pa.enter_context(tc.tile_pool(name="wpool", bufs=1))
            xp = pa.enter_context(tc.tile_pool(name="xpool", bufs=2))
            ep = pa.enter_context(tc.tile_pool(name="aeps", bufs=2))
            pp = pa.enter_context(tc.tile_pool(name="apsum", bufs=2, space="PSUM"))
            tp = pa.enter_context(tc.tile_pool(name="atpsum", bufs=2, space="PSUM"))

            wq_sb = wp.tile([128, NF, DQ], F32R)
            wkv_sb = wp.tile([128, NF, 512], F32R)
            NSPLIT = 4
            for sp in range(NSPLIT):
                f0, f1 = sp * NF // NSPLIT, (sp + 1) * NF // NSPLIT
                nc.sync.dma_start(
                    wq_sb[:, f0:f1, :],
                    wq[f0 * 128:f1 * 128, :].rearrange(
                        "(f p) q -> p f q", p=128))
                nc.sync.dma_start(
                    wkv_sb[:, f0:f1, :],
                    wkv[f0 * 128:f1 * 128, :].rearrange(
                        "(f p) q -> p f q", p=128))

            def emit_chunk_mms(ch):
                x_sb = xp.tile([128, NF, CH], F32R, tag="x")
                nc.sync.dma_start(
                    x_sb[:],
                    xT[:, ch * CH:(ch + 1) * CH].rearrange("(f p) t -> p f t", p=128))
                ps_q = pp.tile([128, DQ], F32, tag="psq")
                ps_kv = pp.tile([128, 512], F32, tag="pskv")
                for f in range(NF):
                    lhs = x_sb[:, f, :]
                    nc.tensor.matmul(ps_q[:], lhs, wq_sb[:, f, :],
                                     start=(f == 0), stop=(f == NF - 1))
                    nc.tensor.matmul(ps_kv[:], lhs, wkv_sb[:, f, :],
                                     start=(f == 0), stop=(f == NF - 1))
                return ps_q, ps_kv

            def emit_chunk_dve(ch, ps_q, ps_kv):
                tg = ch * CH
                cos_t = ep.tile([128, 128], F32, tag="cos")
                nc.sync.dma_start(cos_t[:], cosd[tg:tg + 128, :])
                sin_t = ep.tile([128, 128], F32, tag="sin")
                nc.sync.dma_start(sin_t[:], sind[tg:tg + 128, :])

                # Q rope (2 heads)
                stage_q = ep.tile([128, DQ], F32R, tag="stq")
                tmp1 = ep.tile([128, 128], F32, tag="tmp1")
                tmp2 = ep.tile([128, 128], F32, tag="tmp2")
                for h in range(2):
                    fi = ps_q[:, h * 256:h * 256 + 128]
                    se = ps_q[:, h * 256 + 128:h * 256 + 256]
                    nc.vector.tensor_mul(tmp1[:], fi, cos_t[:])
                    nc.vector.tensor_mul(tmp2[:], se, sin_t[:])
                    nc.vector.tensor_sub(stage_q[:, h * 256:h * 256 + 128],
                                         tmp1[:], tmp2[:])
                    nc.vector.tensor_mul(tmp1[:], se, cos_t[:])
                    nc.vector.tensor_mul(tmp2[:], fi, sin_t[:])
                    nc.vector.tensor_add(
                        stage_q[:, h * 256 + 128:h * 256 + 256],
                        tmp1[:], tmp2[:])

                # K: rmsnorm + scale + rope
                sq = ep.tile([128, 256], F32, tag="sq")
                ssq = ep.tile([128, 1], F32, tag="ssq")
                nc.scalar.activation(sq[:], ps_kv[:, 0:256], AF.Square,
                                     bias=zero_b[:], accum_out=ssq[:])
                std = ep.tile([128, 1], F32, tag="std")
                nc.scalar.activation(std[:], ssq[:], AF.Sqrt,
                                     bias=eps_b[:], scale=1.0 / 256.0)
                rstd = ep.tile([128, 1], F32, tag="rstd")
                nc.vector.reciprocal(rstd[:], std[:])
                kn = ep.tile([128, 256], F32, tag="kn")
                nc.vector.tensor_scalar_mul(kn[:], ps_kv[:, 0:256], rstd[:])
                kn2 = ep.tile([128, 256], F32, tag="kn2")
                nc.vector.tensor_mul(kn2[:], kn[:], ksc[:])
                stage_k = ep.tile([128, 256], F32R, tag="stk")
                nc.vector.tensor_mul(tmp1[:], kn2[:, 0:128], cos_t[:])
                nc.vector.tensor_mul(tmp2[:], kn2[:, 128:256], sin_t[:])
                nc.vector.tensor_sub(stage_k[:, 0:128], tmp1[:], tmp2[:])
                nc.vector.tensor_mul(tmp1[:], kn2[:, 128:256], cos_t[:])
                nc.vector.tensor_mul(tmp2[:], kn2[:, 0:128], sin_t[:])
                nc.vector.tensor_add(stage_k[:, 128:256], tmp1[:], tmp2[:])

                # V: rmsnorm + scale
                sqv = ep.tile([128, 256], F32, tag="sqv")
                ssqv = ep.tile([128, 1], F32, tag="ssqv")
                nc.scalar.activation(sqv[:], ps_kv[:, 256:512], AF.Square,
                                     bias=zero_b[:], accum_out=ssqv[:])
                stdv = ep.tile([128, 1], F32, tag="stdv")
                nc.scalar.activation(stdv[:], ssqv[:], AF.Sqrt,
                                     bias=eps_b[:], scale=1.0 / 256.0)
                rstdv = ep.tile([128, 1], F32, tag="rstdv")
                nc.vector.reciprocal(rstdv[:], stdv[:])
                vn = ep.tile([128, 256], F32, tag="vn")
                nc.vector.tensor_scalar_mul(vn[:], ps_kv[:, 256:512], rstdv[:])
                stage_v = ep.tile([128, 256], F32R, tag="stv")
                nc.vector.tensor_mul(stage_v[:], vn[:], vsc[:])
                nc.sync.dma_start(vd[tg:tg + 128, :], stage_v[:])
                return stage_q, stage_k

            def emit_chunk_transposes(ch, stage_q, stage_k):
                tg = ch * CH
                tq_ps = tp.tile([128, 512], F32, tag="tq")
                for dblk in range(4):
                    nc.tensor.matmul(
                        tq_ps[:, dblk * 128:(dblk + 1) * 128].bitcast(F32R),
                        stage_q[:, dblk * 128:(dblk + 1) * 128],
                        ident[:], is_transpose=True)
                qt_stage = ep.tile([128, 512], F32R, tag="qts")
                nc.vector.tensor_copy(qt_stage[:], tq_ps[:].bitcast(F32R))
                nc.sync.dma_start(
                    qTd[:, tg:tg + 128].rearrange("(i p) t -> p i t", p=128),
                    qt_stage[:].rearrange("p (i t) -> p i t", i=4))
                tk_ps = tp.tile([128, 256], F32, tag="tk")
                for dblk in range(2):
                    nc.tensor.matmul(
                        tk_ps[:, dblk * 128:(dblk + 1) * 128].bitcast(F32R),
                        stage_k[:, dblk * 128:(dblk + 1) * 128],
                        ident[:], is_transpose=True)
                kt_stage = ep.tile([128, 256], F32R, tag="kts")
                nc.vector.tensor_copy(kt_stage[:], tk_ps[:].bitcast(F32R))
                nc.sync.dma_start(
                    kTd[:, tg:tg + 128].rearrange("(i p) t -> p i t", p=128),
                    kt_stage[:].rearrange("p (i t) -> p i t", i=2))

            pending = None
            for ch in range(NCH):
                ps_q, ps_kv = emit_chunk_mms(ch)
                stages = emit_chunk_dve(ch, ps_q, ps_kv)
                if pending is not None:
                    emit_chunk_transposes(pending[0], pending[1], pending[2])
                pending = (ch, stages[0], stages[1])
            emit_chunk_transposes(pending[0], pending[1], pending[2])

        # ---------------- Phases B/C: attention ----------------
        with ExitStack() as pot:
            otp = pot.enter_context(tc.tile_pool(name="otpool", bufs=1))
            OT = otp.tile([128, 4, T], F32R)    # O^T, d-tile major

            with ExitStack() as pbc:
              kvp = pbc.enter_context(tc.tile_pool(name="kvpool", bufs=1))
              qp = pbc.enter_context(tc.tile_pool(name="qpool", bufs=2))
              for bb in (range(b) if "B" in phases else []):
                  with ExitStack() as pb:
                      kT = kvp.tile([128, 2, s], F32R, tag="kT")
                      nc.sync.dma_start(
                          kT[:],
                          kTd[:, bb * s:(bb + 1) * s].rearrange(
                              "(i p) t -> p i t", p=128))
                      v_sb = kvp.tile([128, TTB, 256], F32R, tag="v")
                      nc.sync.dma_start(
                          v_sb[:],
                          vd[bb * s:(bb + 1) * s, :].rearrange(
                              "(n p) q -> p n q", p=128))

                      for h in (range(2) if "C" in phases else []):
                          with ExitStack() as ph:
                              qT = qp.tile([128, 2, s], F32R, tag="qT")
                              nc.sync.dma_start(
                                  qT[:],
                                  qTd[h * 256:(h + 1) * 256,
                                      bb * s:(bb + 1) * s].rearrange(
                                      "(i p) t -> p i t", p=128))

                              with ExitStack() as pc:
                                  spool = pc.enter_context(
                                      tc.tile_pool(name="spsum", bufs=2, space="PSUM"))
                                  opool = pc.enter_context(
                                      tc.tile_pool(name="opsum", bufs=2, space="PSUM"))
                                  rpool = pc.enter_context(
                                      tc.tile_pool(name="rpsum", bufs=1, space="PSUM"))
                                  ptp = pc.enter_context(
                                      tc.tile_pool(name="ptpool", bufs=4))
                                  rcp = pc.enter_context(
                                      tc.tile_pool(name="rcpool", bufs=2))

                                  for tqb in range(NQB):
                                      jmax = 4 * tqb + 4
                                      o_ps0 = opool.tile([128, 512], F32, tag="o0")
                                      o_ps1 = opool.tile([128, 512], F32, tag="o1")
                                      rb_ps = rpool.tile([128, 512], F32)
                                      qsl0 = qT[:, 0, tqb * 512:(tqb + 1) * 512]
                                      qsl1 = qT[:, 1, tqb * 512:(tqb + 1) * 512]

                                      def emit_s(j):
                                          s_ps = spool.tile(
                                              [128, 512], F32, tag="s")
                                          nc.tensor.matmul(
                                              s_ps[:],
                                              kT[:, 0, j * 128:(j + 1) * 128],
                                              qsl0, start=True, stop=False)
                                          nc.tensor.matmul(
                                              s_ps[:],
                                              kT[:, 1, j * 128:(j + 1) * 128],
                                              qsl1, start=False, stop=True)
                                          if j >= 4 * tqb:
                                              nc.vector.tensor_add(
                                                  s_ps[:], s_ps[:],
                                                  maskb[:, j - 4 * tqb, :])
                                          return s_ps

                                      # software pipeline: S_{j+1} overlaps exp_j
                                      s_cur = emit_s(0)
                                      for j in range(jmax):
                                          pT = ptp.tile([128, 512], F32R)
                                          nc.scalar.activation(
                                              pT[:], s_cur[:], AF.Exp,
                                              bias=zero_b[:], scale=0.0625)
                                          if j + 1 < jmax:
                                              s_cur = emit_s(j + 1)
                                          nc.tensor.matmul(
                                              rb_ps[:], ones_sb[:], pT[:],
                                              start=(j == 0), stop=(j == jmax - 1),
                                              skip_group_check=True)
                                          nc.tensor.matmul(
                                              o_ps0[:], v_sb[:, j, 0:128], pT[:],
                                              start=(j == 0), stop=(j == jmax - 1),
                                              skip_group_check=True)
                                          nc.tensor.matmul(
                                              o_ps1[:], v_sb[:, j, 128:256], pT[:],
                                              start=(j == 0), stop=(j == jmax - 1),
                                              skip_group_check=True)
                                      recip = rcp.tile([128, 512], F32)
                                      nc.vector.reciprocal(recip[:], rb_ps[:])
                                      nc.vector.tensor_mul(
                                          OT[:, 2 * h, bb * s + tqb * 512:
                                             bb * s + (tqb + 1) * 512],
                                          o_ps0[:], recip[:])
                                      nc.vector.tensor_mul(
                                          OT[:, 2 * h + 1, bb * s + tqb * 512:
                                             bb * s + (tqb + 1) * 512],
                                          o_ps1[:], recip[:])

            # ---------------- Phase D: output projection ----------------
            if "D" in phases:
                with ExitStack() as pd:
                    wop = pd.enter_context(tc.tile_pool(name="wopool", bufs=1))
                    wo_sb = wop.tile([128, 4, d], F32R)
                    for g in range(4):
                        nc.sync.dma_start(
                            wo_sb[:, g, :], wo[g * 128:(g + 1) * 128, :])
                    ysp = pd.enter_context(tc.tile_pool(name="ypool", bufs=8))
                    yps = pd.enter_context(
                        tc.tile_pool(name="ypsum", bufs=4, space="PSUM"))
                    for tt in range(T // 128):
                        for eb in range(d // 512):
                            y_ps = yps.tile([128, 512], F32)
                            for g in range(4):
                                nc.tensor.matmul(
                                    y_ps[:], OT[:, g, tt * 128:(tt + 1) * 128],
                                    wo_sb[:, g, eb * 512:(eb + 1) * 512],
                                    start=(g == 0), stop=(g == 3))
                            y_sb = ysp.tile([128, 512], F32)
                            nc.scalar.copy(y_sb[:], y_ps[:])
                            nc.sync.dma_start(
                                y[tt * 128:(tt + 1) * 128,
                                  eb * 512:(eb + 1) * 512],
                                y_sb[:])

    if legalize:
        legalize_waits(nc)
    return nc


def host_inputs(x, Wq, Wk, Wv, Wo, k_scale, v_scale, position, core,
                b=B, s=S, d=D):
    """Build the per-core input map (numpy, float32 where dtype is f32r)."""
    T = b * s
    xT = np.ascontiguousarray(x.reshape(T, d).T)

    pos = position.reshape(T).astype(np.float32)
    j = np.arange(128, dtype=np.float32)
    timescale = ROPE_BASE ** (2.0 * j / HD)
    ang = pos[:, None] / timescale[None, :]
    cosd = np.cos(ang).astype(np.float32)
    sind = np.sin(ang).astype(np.float32)

    ksc = np.broadcast_to((1.0 + k_scale).astype(np.float32), (128, 256)).copy()
    vsc = np.broadcast_to((1.0 + v_scale).astype(np.float32), (128, 256)).copy()
    ident = np.eye(128, dtype=np.float32)
    ones = np.ones((128, 128), dtype=np.float32)
    maskd = np.empty((4, 128, 512), dtype=np.float32)
    for o in range(4):
        p = np.arange(128)[:, None] + 128 * o
        c = np.arange(512)[None, :]
        maskd[o] = np.where(p <= c, 0.0, MASKV).astype(np.float32)

    wq_c = np.ascontiguousarray(Wq[:, core * DQ:(core + 1) * DQ])
    wkv_c = np.concatenate(
        [Wk[:, core * 256:(core + 1) * 256],
         Wv[:, core * 256:(core + 1) * 256]], axis=1)
    wo_c = np.ascontiguousarray(Wo[core * DQ:(core + 1) * DQ, :])

    return {
        "xT": xT.astype(np.float32), "wq": wq_c.astype(np.float32),
        "wkv": wkv_c.astype(np.float32), "wo": wo_c.astype(np.float32),
        "cosd": cosd, "sind": sind, "kscd": ksc, "vscd": vsc,
        "identd": ident, "onesd": ones, "maskd": maskd,
    }


def kernel(x, Wq, Wk, Wv, Wo, k_scale, v_scale, mask, position):
    from concourse.bass_utils import run_bass_kernel_spmd
    b, s, d = x.shape
    nc = build_bass(b=b, s=s, d=d)
    in_maps = [
        host_inputs(x, Wq, Wk, Wv, Wo, k_scale, v_scale, position, core,
                    b=b, s=s, d=d)
        for core in range(N_CORES)
    ]
    res = run_bass_kernel_spmd(nc, in_maps, list(range(N_CORES)))
    out = None
    for r in res.results:
        yc = r["y"]
        out = yc if out is None else out + yc
    return out.reshape(b, s, d).astype(np.float32)



# revision 6
# speedup vs baseline: 1.1677x; 1.1677x over previous
"""Tensor-parallel MultiHeadAttention (GQA + RMSNorm-KV + RoPE) for 8 trn2 cores.

Sharding: KV head h -> core h (HKV=8); Q heads {2h, 2h+1}; x replicated;
Wo row-sharded; host sums the 8 partial outputs.

v2: all-bf16 matmul I/O, SBUF-resident q/k/v/O (no DRAM round trips),
Q projected directly in transposed layout, K transposed via DMA XBAR,
causal column-trim on diagonal blocks, pipelined startup DMA.
"""
import sys
sys.path.insert(0, '/opt/trn_rl_repo')
import numpy as np
import ml_dtypes
import concourse.bass as bass
import concourse.tile as tile
from concourse import mybir
from contextlib import ExitStack

F32 = mybir.dt.float32
BF16 = mybir.dt.bfloat16
AF = mybir.ActivationFunctionType
_BF16NP = np.dtype(ml_dtypes.bfloat16)

# Problem constants (full size).
B = 2
S = 2048          # sequence per batch
D = 4096          # model dim
HD = 256          # head dim
DQ = 512          # per-core q width (2 heads)
ROPE_BASE = 10000.0
MASKV = -1e10
EPS = 1e-6
N_CORES = 8
CH = 256          # phase-A token chunk


def legalize_waits(nc, max_waits=1):
    """This container's walrus encodes at most one sem-wait per instruction.
    Move extra waits onto same-engine NOPs placed just before (engine FIFO
    order makes that equivalent)."""
    n = 0
    for f in nc.m.functions:
        for blk in f.blocks:
            out = []
            for ins in blk.instructions:
                si = ins.sync_info
                if si is not None and si.on_wait and len(si.on_wait) > max_waits:
                    waits = list(si.on_wait)
                    for w in waits[max_waits:]:
                        nop = mybir.InstNoOp(name=nc.get_next_instruction_name())
                        nop.engine = ins.engine
                        nop.sync_info = mybir.SyncInfo(on_wait=[w], on_update=[])
                        out.append(nop)
                    ins.sync_info = mybir.SyncInfo(
                        on_wait=waits[:max_waits], on_update=list(si.on_update or []))
                    n += 1
                out.append(ins)
            blk.instructions.clear()
            for i in out:
                blk.instructions.append(i)
    return n


def build_bass(b=B, s=S, d=D, legalize=True, phases="ABCD", debug=False):
    T = b * s              # total tokens
    NF = d // 128          # contraction tiles
    NCH = T // CH          # phase-A chunks
    NQB = s // 512         # 512-wide query blocks per batch

    nc = bass.Bass()
    if debug:
        dbg_qT = nc.dram_tensor("dbg_qT", [128, 4, T], BF16, kind="ExternalOutput")
        dbg_kT = nc.dram_tensor("dbg_kT", [128, 2, T], BF16, kind="ExternalOutput")
        dbg_v = nc.dram_tensor("dbg_v", [128, T // 128, 256], BF16,
                               kind="ExternalOutput")
        dbg_ot = nc.dram_tensor("dbg_ot", [128, 4, T], BF16, kind="ExternalOutput")
    xT = nc.dram_tensor("xT", [d, T], BF16, kind="ExternalInput")
    wq = nc.dram_tensor("wq", [d, DQ], BF16, kind="ExternalInput")
    wkv = nc.dram_tensor("wkv", [d, 512], BF16, kind="ExternalInput")
    wo = nc.dram_tensor("wo", [DQ, d], BF16, kind="ExternalInput")
    cosTd = nc.dram_tensor("cosTd", [128, T], BF16, kind="ExternalInput")
    sinTd = nc.dram_tensor("sinTd", [128, T], BF16, kind="ExternalInput")
    kscd = nc.dram_tensor("kscd", [128, 256], F32, kind="ExternalInput")
    vscd = nc.dram_tensor("vscd", [128, 256], F32, kind="ExternalInput")
    onesd = nc.dram_tensor("onesd", [128, 128], BF16, kind="ExternalInput")
    trid = nc.dram_tensor("trid", [128, 128], F32, kind="ExternalInput")
    y = nc.dram_tensor("y", [T, d], BF16, kind="ExternalOutput")

    with tile.TileContext(nc) as tc, ExitStack() as top:
        cp = top.enter_context(tc.tile_pool(name="const", bufs=1))
        zero_b = cp.tile([128, 1], F32)
        nc.vector.memset(zero_b[:], 0.0)
        eps_b = cp.tile([128, 1], F32)
        nc.vector.memset(eps_b[:], EPS)
        ksc = cp.tile([128, 256], F32)
        vsc = cp.tile([128, 256], F32)
        ones_sb = cp.tile([128, 128], BF16)
        tri = cp.tile([128, 128], F32)

        pk = top.enter_context(tc.tile_pool(name="qkvpool", bufs=1))
        qT_all = pk.tile([128, 4, T], BF16)   # [j, dq-128-block, token]
        kT_all = pk.tile([128, 2, T], BF16)   # [j, d-128-block, token]
        v_all = pk.tile([128, T // 128, 256], BF16)  # [token%128, tile, dim]

        # ---------------- Phase A: projections + norm + rope ----------------
        if "A" in phases:
          with ExitStack() as pa:
            rp = pa.enter_context(tc.tile_pool(name="ropetab", bufs=1))
            cosT = rp.tile([128, T], BF16)
            sinT = rp.tile([128, T], BF16)
            wp = pa.enter_context(tc.tile_pool(name="wpool", bufs=1))
            wq_sb = wp.tile([128, NF, DQ], BF16)
            wkv_sb = wp.tile([128, NF, 512], BF16)
            xp = pa.enter_context(tc.tile_pool(name="xpool", bufs=2))
            ep = pa.enter_context(tc.tile_pool(name="aeps", bufs=2))
            pp = pa.enter_context(tc.tile_pool(name="apsum", bufs=2, space="PSUM"))

            # Startup DMA order: first x half-chunk, then weight splits
            # interleaved so the PE can start after ~3 tiles have landed.
            x0 = xp.tile([128, NF, CH], BF16, tag="x")
            nc.sync.dma_start(
                x0[:, 0:NF // 2, :],
                xT[0:d // 2, 0:CH].rearrange("(f p) t -> p f t", p=128))

            WS = 4  # f-tiles per weight split

            def dma_w(si):
                f0, f1 = si * WS, (si + 1) * WS
                nc.sync.dma_start(
                    wq_sb[:, f0:f1, :],
                    wq[f0 * 128:f1 * 128, :].rearrange("(f p) q -> p f q", p=128))
                nc.sync.dma_start(
                    wkv_sb[:, f0:f1, :],
                    wkv[f0 * 128:f1 * 128, :].rearrange("(f p) q -> p f q", p=128))

            for si in range(4):
                dma_w(si)
            nc.sync.dma_start(
                x0[:, NF // 2:, :],
                xT[d // 2:d, 0:CH].rearrange("(f p) t -> p f t", p=128))
            for si in range(4, NF // WS):
                dma_w(si)
            # small consts ride along here (DMA has slack now)
            nc.sync.dma_start(ksc[:], kscd[:])
            nc.sync.dma_start(vsc[:], vscd[:])
            nc.sync.dma_start(ones_sb[:], onesd[:])
            nc.sync.dma_start(tri[:], trid[:])

            x_tiles = {0: x0}

            def emit_chunk(c):
                tg = c * CH
                # prefetch next x chunk
                if c + 1 < NCH:
                    xn = xp.tile([128, NF, CH], BF16, tag="x")
                    nc.sync.dma_start(
                        xn[:],
                        xT[:, (c + 1) * CH:(c + 2) * CH].rearrange(
                            "(f p) t -> p f t", p=128))
                    x_tiles[c + 1] = xn
                if c == 0:
                    # rope tables land while chunk-0 matmuls run
                    nc.sync.dma_start(cosT[:], cosTd[:])
                    nc.sync.dma_start(sinT[:], sinTd[:])
                x_sb = x_tiles.pop(c)

                psA = pp.tile([128, 512], F32, tag="psA")  # q dqb0|dqb1
                psB = pp.tile([128, 512], F32, tag="psB")  # q dqb2|dqb3
                kv0 = pp.tile([128, 512], F32, tag="kv0")  # tokens 0:128
                kv1 = pp.tile([128, 512], F32, tag="kv1")  # tokens 128:256
                for f in range(NF):
                    st, en = (f == 0), (f == NF - 1)
                    for dqb in range(4):
                        pst = psA if dqb < 2 else psB
                        col = (dqb % 2) * CH
                        # start=True zeroes the WHOLE PSUM bank, so only the
                        # first group in each bank may use it; the second
                        # accumulates onto the zeroed half from f=0 on.
                        nc.tensor.matmul(
                            pst[:, col:col + CH],
                            wq_sb[:, f, dqb * 128:(dqb + 1) * 128],
                            x_sb[:, f, :], start=(st and col == 0), stop=en,
                            skip_group_check=True)
                    nc.tensor.matmul(kv0[:], x_sb[:, f, 0:128],
                                     wkv_sb[:, f, :], start=st, stop=en)
                    nc.tensor.matmul(kv1[:], x_sb[:, f, 128:256],
                                     wkv_sb[:, f, :], start=st, stop=en)

                # Q rope (transposed layout: partition = freq index j)
                cs = cosT[:, tg:tg + CH]
                sn = sinT[:, tg:tg + CH]
                for hp in range(2):
                    ps = psA if hp == 0 else psB
                    fi, se = ps[:, 0:CH], ps[:, CH:2 * CH]
                    t1 = ep.tile([128, CH], F32, tag="t1")
                    t2 = ep.tile([128, CH], F32, tag="t2")
                    nc.vector.tensor_mul(t1[:], fi, cs)
                    nc.vector.tensor_mul(t2[:], se, sn)
                    nc.vector.tensor_sub(qT_all[:, 2 * hp, tg:tg + CH],
                                         t1[:], t2[:])
                    nc.vector.tensor_mul(t1[:], se, cs)
                    nc.vector.tensor_mul(t2[:], fi, sn)
                    nc.vector.tensor_add(qT_all[:, 2 * hp + 1, tg:tg + CH],
                                         t1[:], t2[:])

                # K: rmsnorm+scale -> XBAR transpose -> rope; V: rmsnorm+scale
                for sub in range(2):
                    kv = kv0 if sub == 0 else kv1
                    ts_ = tg + sub * 128
                    sq = ep.tile([128, 256], F32, tag="sq")
                    ssq = ep.tile([128, 1], F32, tag="ssq")
                    nc.scalar.activation(sq[:], kv[:, 0:256], AF.Square,
                                         bias=zero_b[:], accum_out=ssq[:])
                    std = ep.tile([128, 1], F32, tag="std")
                    nc.scalar.activation(std[:], ssq[:], AF.Sqrt,
                                         bias=eps_b[:], scale=1.0 / 256.0)
                    rstd = ep.tile([128, 1], F32, tag="rstd")
                    nc.vector.reciprocal(rstd[:], std[:])
                    kn = ep.tile([128, 256], F32, tag="kn")
                    nc.vector.tensor_scalar_mul(kn[:], kv[:, 0:256], rstd[:])
                    stage_k = ep.tile([128, 256], BF16, tag="stk")
                    nc.vector.tensor_mul(stage_k[:], kn[:], ksc[:])
                    kp0 = ep.tile([128, 128], BF16, tag="kp0")
                    kp1 = ep.tile([128, 128], BF16, tag="kp1")
                    nc.sync.dma_start(kp0[:], stage_k[:, 0:128], transpose=True)
                    nc.sync.dma_start(kp1[:], stage_k[:, 128:256], transpose=True)
                    csk = cosT[:, ts_:ts_ + 128]
                    snk = sinT[:, ts_:ts_ + 128]
                    b1 = ep.tile([128, 128], BF16, tag="b1")
                    b2 = ep.tile([128, 128], BF16, tag="b2")
                    nc.vector.tensor_mul(b1[:], kp0[:], csk)
                    nc.vector.tensor_mul(b2[:], kp1[:], snk)
                    nc.vector.tensor_sub(kT_all[:, 0, ts_:ts_ + 128],
                                         b1[:], b2[:])
                    nc.vector.tensor_mul(b1[:], kp1[:], csk)
                    nc.vector.tensor_mul(b2[:], kp0[:], snk)
                    nc.vector.tensor_add(kT_all[:, 1, ts_:ts_ + 128],
                                         b1[:], b2[:])

                    sqv = ep.tile([128, 256], F32, tag="sqv")
                    ssqv = ep.tile([128, 1], F32, tag="ssqv")
                    nc.scalar.activation(sqv[:], kv[:, 256:512], AF.Square,
                                         bias=zero_b[:], accum_out=ssqv[:])
                    stdv = ep.tile([128, 1], F32, tag="stdv")
                    nc.scalar.activation(stdv[:], ssqv[:], AF.Sqrt,
                                         bias=eps_b[:], scale=1.0 / 256.0)
                    rstdv = ep.tile([128, 1], F32, tag="rstdv")
                    nc.vector.reciprocal(rstdv[:], stdv[:])
                    vn = ep.tile([128, 256], F32, tag="vn")
                    nc.vector.tensor_scalar_mul(vn[:], kv[:, 256:512], rstdv[:])
                    nc.vector.tensor_mul(v_all[:, ts_ // 128, :], vn[:], vsc[:])

            for c in range(NCH):
                emit_chunk(c)
            if debug:
                nc.sync.dma_start(dbg_qT[:], qT_all[:])
                nc.sync.dma_start(dbg_kT[:], kT_all[:])
                nc.sync.dma_start(dbg_v[:], v_all[:])

        # ---------------- Phases B/C/D ----------------
        with ExitStack() as pcd:
            otp = pcd.enter_context(tc.tile_pool(name="otpool", bufs=1))
            OT = otp.tile([128, 4, T], BF16)    # O^T, d-tile major
            wop = pcd.enter_context(tc.tile_pool(name="wopool", bufs=1))
            wo_sb = wop.tile([128, 4, d], BF16)
            if "D" in phases:
                for g in range(4):
                    nc.sync.dma_start(
                        wo_sb[:, g, :], wo[g * 128:(g + 1) * 128, :])

            for bb in (range(b) if "C" in phases else []):
                for h in range(2):
                    with ExitStack() as pc:
                        spool = pc.enter_context(
                            tc.tile_pool(name="spsum", bufs=2, space="PSUM"))
                        opool = pc.enter_context(
                            tc.tile_pool(name="opsum", bufs=2, space="PSUM"))
                        rpool = pc.enter_context(
                            tc.tile_pool(name="rpsum", bufs=1, space="PSUM"))
                        ptp = pc.enter_context(
                            tc.tile_pool(name="ptpool", bufs=4))
                        rcp = pc.enter_context(
                            tc.tile_pool(name="rcpool", bufs=2))

                        for tqb in range(NQB):
                            jmax = 4 * tqb + 4
                            Q0 = bb * s + tqb * 512
                            o_ps0 = opool.tile([128, 512], F32, tag="o0")
                            o_ps1 = opool.tile([128, 512], F32, tag="o1")
                            rb_ps = rpool.tile([128, 512], F32)

                            def emit_s(j):
                                diag = j - 4 * tqb
                                c0 = max(0, diag * 128)
                                s_ps = spool.tile([128, 512], F32, tag="s")
                                k0 = bb * s + j * 128
                                nc.tensor.matmul(
                                    s_ps[:, c0:512],
                                    kT_all[:, 0, k0:k0 + 128],
                                    qT_all[:, 2 * h, Q0 + c0:Q0 + 512],
                                    start=True, stop=False,
                                    skip_group_check=True)
                                nc.tensor.matmul(
                                    s_ps[:, c0:512],
                                    kT_all[:, 1, k0:k0 + 128],
                                    qT_all[:, 2 * h + 1, Q0 + c0:Q0 + 512],
                                    start=False, stop=True,
                                    skip_group_check=True)
                                if diag >= 0:
                                    nc.vector.tensor_add(
                                        s_ps[:, c0:c0 + 128],
                                        s_ps[:, c0:c0 + 128], tri[:])
                                return s_ps, c0

                            # software pipeline: S_{j+1} overlaps exp_j
                            s_cur, c_cur = emit_s(0)
                            for j in range(jmax):
                                pT = ptp.tile([128, 512], BF16)
                                nc.scalar.activation(
                                    pT[:, c_cur:512], s_cur[:, c_cur:512],
                                    AF.Exp, bias=zero_b[:], scale=0.0625)
                                c_j = c_cur
                                if j + 1 < jmax:
                                    s_cur, c_cur = emit_s(j + 1)
                                vi = bb * (s // 128) + j
                                nc.tensor.matmul(
                                    rb_ps[:, c_j:512], ones_sb[:],
                                    pT[:, c_j:512],
                                    start=(j == 0), stop=(j == jmax - 1),
                                    skip_group_check=True)
                                nc.tensor.matmul(
                                    o_ps0[:, c_j:512], v_all[:, vi, 0:128],
                                    pT[:, c_j:512],
                                    start=(j == 0), stop=(j == jmax - 1),
                                    skip_group_check=True)
                                nc.tensor.matmul(
                                    o_ps1[:, c_j:512], v_all[:, vi, 128:256],
                                    pT[:, c_j:512],
                                    start=(j == 0), stop=(j == jmax - 1),
                                    skip_group_check=True)
                            recip = rcp.tile([128, 512], F32)
                            nc.vector.reciprocal(recip[:], rb_ps[:])
                            nc.vector.tensor_mul(
                                OT[:, 2 * h, Q0:Q0 + 512],
                                o_ps0[:], recip[:])
                            nc.vector.tensor_mul(
                                OT[:, 2 * h + 1, Q0:Q0 + 512],
                                o_ps1[:], recip[:])

            if debug:
                nc.sync.dma_start(dbg_ot[:], OT[:])

            # ---------------- Phase D: output projection ----------------
            if "D" in phases:
                with ExitStack() as pd:
                    ysp = pd.enter_context(tc.tile_pool(name="ypool", bufs=2))
                    yps = pd.enter_context(
                        tc.tile_pool(name="ypsum", bufs=2, space="PSUM"))
                    for tt in range(T // 128):
                        y_row = ysp.tile([128, d], BF16, tag="yrow")
                        for eb in range(d // 512):
                            y_ps = yps.tile([128, 512], F32, tag="yps")
                            for g in range(4):
                                nc.tensor.matmul(
                                    y_ps[:], OT[:, g, tt * 128:(tt + 1) * 128],
                                    wo_sb[:, g, eb * 512:(eb + 1) * 512],
                                    start=(g == 0), stop=(g == 3))
                            nc.scalar.copy(y_row[:, eb * 512:(eb + 1) * 512],
                                           y_ps[:])
                        nc.sync.dma_start(
                            y[tt * 128:(tt + 1) * 128, :], y_row[:])

    if legalize:
        legalize_waits(nc)
    return nc


_shared_cache = {}


def _host_shared(x, position, b=B, s=S, d=D):
    key = (id(x), id(position))
    if key in _shared_cache:
        return _shared_cache[key]
    T = b * s
    xT = np.ascontiguousarray(x.reshape(T, d).T).astype(_BF16NP)

    pos = position.reshape(T).astype(np.float64)
    j = np.arange(128, dtype=np.float64)
    timescale = ROPE_BASE ** (2.0 * j / HD)
    ang = pos[None, :] / timescale[:, None]          # [128, T]
    cosT = np.cos(ang).astype(_BF16NP)
    sinT = np.sin(ang).astype(_BF16NP)

    ones = np.ones((128, 128), dtype=_BF16NP)
    tri = np.where(np.arange(128)[:, None] <= np.arange(128)[None, :],
                   0.0, MASKV).astype(np.float32)
    out = {"xT": xT, "cosTd": cosT, "sinTd": sinT, "onesd": ones, "trid": tri}
    _shared_cache.clear()
    _shared_cache[key] = out
    return out


def host_inputs(x, Wq, Wk, Wv, Wo, k_scale, v_scale, position, core,
                b=B, s=S, d=D):
    """Build the per-core input map."""
    shared = _host_shared(x, position, b=b, s=s, d=d)
    ksc = np.broadcast_to((1.0 + k_scale).astype(np.float32), (128, 256)).copy()
    vsc = np.broadcast_to((1.0 + v_scale).astype(np.float32), (128, 256)).copy()
    wq_c = np.ascontiguousarray(Wq[:, core * DQ:(core + 1) * DQ]).astype(_BF16NP)
    wkv_c = np.concatenate(
        [Wk[:, core * 256:(core + 1) * 256],
         Wv[:, core * 256:(core + 1) * 256]], axis=1).astype(_BF16NP)
    wo_c = np.ascontiguousarray(Wo[core * DQ:(core + 1) * DQ, :]).astype(_BF16NP)
    return {
        **shared,
        "wq": wq_c, "wkv": wkv_c, "wo": wo_c,
        "kscd": ksc, "vscd": vsc,
    }


def kernel(x, Wq, Wk, Wv, Wo, k_scale, v_scale, mask, position):
    from concourse.bass_utils import run_bass_kernel_spmd
    b, s, d = x.shape
    nc = build_bass(b=b, s=s, d=d)
    in_maps = [
        host_inputs(x, Wq, Wk, Wv, Wo, k_scale, v_scale, position, core,
                    b=b, s=s, d=d)
        for core in range(N_CORES)
    ]
    res = run_bass_kernel_spmd(nc, in_maps, list(range(N_CORES)))
    out = None
    for r in res.results:
        yc = np.asarray(r["y"]).astype(np.float32)
        out = yc if out is None else out + yc
    return out.reshape(b, s, d).astype(np.float32)
